# revision 59
# baseline (speedup 1.0000x reference)
"""CeNN front-end Trainium2 kernel.

Reference computation (per batch image u [1,H,W]):
    control = conv3x3_same(u, W_B) + 0                     # [64,H,W]
    x0 = control
    x_{k+1} = alpha*x_k + beta*(conv3x3_same(tanh(x_k), WA_eff) + control + bias)
    (WA_eff = W_A with diagonal center taps clamped >= 1), 16 steps.

Distribution: 8 cores = (batch b in 0..3) x (H half). Each core owns a
272-row slab (256 valid + 16 halo rows toward the other half). Zero
communication: halo contamination advances one row per step and after 16
steps exactly the 16 halo rows are dirty.

Per-core kernel: channel-major layout [64ch -> partitions, rows, 514 cols
(W+2 zero pad)]. Rows are split into two 64-partition blocks (A on
partitions 0:64, B on 64:128) so elementwise work runs 128 wide.

conv3x3 = 9 accumulating matmuls (K=64 cin, M=64 cout, N=512) at per-tap
free offsets, in bf16 (tanh output + beta-scaled weights).  One more
identity tap adds C = beta*(control+bias) as a single bf16 term.  Four PE
quadrants (tile_position) process four rows concurrently.  The state
update is one fused DVE op per row:
    x' = (x * alpha) + psum        (scalar_tensor_tensor)

Time is blocked T=4 steps per pass over fp16 DRAM ping-pong buffers with
redundant-halo strips.  Pass 0 computes control from u with a K=10 fp32
im2col matmul (9 shifted u copies + ones row for bias).  x_out is fp16,
514-wide padded for contiguous stores; host slices + converts.
"""

import math

import numpy as np
import ml_dtypes

import concourse.bacc as bacc
import concourse.tile as tile
from concourse import mybir
from concourse.bass_utils import run_bass_kernel_spmd

F32 = mybir.dt.float32
F32R = mybir.dt.float32r
F16 = mybir.dt.float16
BF16 = mybir.dt.bfloat16
AF = mybir.ActivationFunctionType
ALU = mybir.AluOpType

FULL_CFG = dict(SLAB=272, HS=68, T=4, NSTEPS=16, RC0=34)


def _derive(cfg):
    d = dict(cfg)
    d["R"] = d["HS"] + 2 * d["T"] + 2          # strip tile rows
    assert d["R"] % 2 == 0
    d["RH"] = d["R"] // 2                      # rows per partition block
    # strips may be ragged: last strip covers the remainder
    strips = []
    o0 = 0
    while o0 < d["SLAB"]:
        hs = min(d["HS"], d["SLAB"] - o0)
        assert hs % 2 == 0
        strips.append((o0, hs))
        o0 += hs
    d["STRIPS"] = strips
    d["NSTRIP"] = len(strips)
    assert d["SLAB"] % d["RC0"] == 0
    d["NCHUNK0"] = d["SLAB"] // d["RC0"]
    assert d["NSTEPS"] % d["T"] == 0
    d["NPASS"] = d["NSTEPS"] // d["T"]
    d.setdefault("DBG_P0_OUT", d["NPASS"] == 0)
    d["UROWS"] = d["SLAB"] + 2
    return d


def build(cfg):
    """Build the per-core Bass program. Returns compiled nc."""
    g = _derive(cfg)
    SLAB, HS, T, RC0 = g["SLAB"], g["HS"], g["T"], g["RC0"]
    R, RH, NSTRIP, NCHUNK0, NPASS, UROWS = (
        g["R"], g["RH"], g["NSTRIP"], g["NCHUNK0"], g["NPASS"], g["UROWS"])
    WP = 514
    W = 512

    nc = bacc.Bacc("TRN2", target_bir_lowering=False, debug=False,
                   num_devices=8)

    u27_in = nc.dram_tensor("u27_in", [27, SLAB, W], BF16,
                            kind="ExternalInput")
    wa_in = nc.dram_tensor("wa_in", [64, 11, 64], F16, kind="ExternalInput")
    wb_in = nc.dram_tensor("wb_in", [27, 64], BF16, kind="ExternalInput")
    nbias_in = nc.dram_tensor("nbias_in", [64, 1], F32, kind="ExternalInput")
    mb_in = nc.dram_tensor("mb_in", [64, 1], F32, kind="ExternalInput")
    alpha_in = nc.dram_tensor("alpha_in", [1, 1], F32, kind="ExternalInput")
    x_out = nc.dram_tensor("x_out", [64, SLAB, WP], F16, kind="ExternalOutput")

    Xd = [nc.dram_tensor(f"Xd{i}", [64, SLAB, WP], F16, kind="Internal")
          for i in range(2)]
    Chi_d = nc.dram_tensor("Chi", [64, SLAB, WP], F16, kind="Internal")

    with tile.TileContext(nc) as tc:
        with tc.tile_pool(name="singles", bufs=1) as singles:
            wa_t = singles.tile([128, 11, 64], F16)
            nc.sync.dma_start(out=wa_t[0:64], in_=wa_in[:, :, :])
            nc.sync.dma_start(out=wa_t[64:128], in_=wa_in[:, :, :])
            wb_t = singles.tile([27, 64], BF16)
            nc.sync.dma_start(out=wb_t, in_=wb_in[:, :])
            nbias_t = singles.tile([128, 1], F32)
            nc.sync.dma_start(out=nbias_t[0:64], in_=nbias_in[:, :])
            nc.sync.dma_start(out=nbias_t[64:128], in_=nbias_in[:, :])
            mbias_t = singles.tile([128, 1], F32)
            nc.sync.dma_start(out=mbias_t[0:64], in_=mb_in[:, :])
            nc.sync.dma_start(out=mbias_t[64:128], in_=mb_in[:, :])
            alpha_t = singles.tile([128, 1], F32)
            nc.sync.dma_start(out=alpha_t, in_=alpha_in[:, :].to_broadcast((128, 1)))
            beta_t = singles.tile([128, 1], F32)
            nc.vector.tensor_scalar(out=beta_t, in0=alpha_t, scalar1=-1.0,
                                    scalar2=1.0, op0=ALU.mult, op1=ALU.add)
            invb_t = singles.tile([128, 1], F32)
            nc.vector.reciprocal(out=invb_t, in_=beta_t)

            # ---------------- pass 0: control -> C only ----------------------
            # One K=27 bf16 matmul per row (u split hi+lo on host, weights
            # hi+lo: uh*wh + ul*wh + uh*wl), two rows concurrent via the two
            # PE column groups.  Only C = beta*(control+bias) is stored;
            # pass 1 derives x0 = C/beta - bias from the loaded C tile.
            with tc.tile_pool(name="p0u", bufs=3) as p0u, \
                 tc.tile_pool(name="p0ps", bufs=4, space="PSUM") as p0ps, \
                 tc.tile_pool(name="p0st", bufs=3) as p0st:
                def p0_load(chk):
                    # host prebuilds the 27-tap im2col (hi/lo split + kh/kw
                    # shifts), so each chunk is a single contiguous DMA --
                    # the sync engine's ~1us per-dma_start issue cost made
                    # 27 small gathers the pass-0 rate limiter.
                    c0 = RC0 * chk
                    u9 = p0u.tile([27, RC0, W], BF16)
                    nc.sync.dma_start(out=u9, in_=u27_in[:, c0:c0 + RC0, :])
                    return u9

                u9s = [p0_load(0), p0_load(1)]
                for chk in range(NCHUNK0):
                    c0 = RC0 * chk
                    if chk + 2 < NCHUNK0:
                        u9s.append(p0_load(chk + 2))
                    u9 = u9s[chk]
                    chst = p0st.tile([64, RC0, WP], F16, tag="chst")
                    nc.vector.memset(chst[:, :, 0:1], 0.0)
                    nc.vector.memset(chst[:, :, 513:514], 0.0)
                    for t in range(0, RC0, 2):
                        pc = p0ps.tile([128, 512], F32)
                        nc.tensor.matmul(pc[0:64, :], wb_t, u9[:, t, :],
                                         start=True, stop=True,
                                         tile_position=(0, 0))
                        nc.tensor.matmul(pc[64:128, :], wb_t, u9[:, t + 1, :],
                                         start=True, stop=True,
                                         skip_group_check=True,
                                         tile_position=(0, 64))
                        # C = beta*psum + beta*bias, split Scalar/DVE
                        nc.scalar.activation(out=chst[:, t, 1:513],
                                             in_=pc[0:64, :],
                                             func=AF.Identity,
                                             scale=beta_t[0:64],
                                             bias=nbias_t[0:64])
                        nc.vector.tensor_scalar(
                            out=chst[:, t + 1, 1:513], in0=pc[64:128, :],
                            scalar1=beta_t[64:128], op0=ALU.mult,
                            scalar2=nbias_t[64:128], op1=ALU.add)
                    nc.sync.dma_start(out=Chi_d[:, c0:c0 + RC0, :],
                                        in_=chst)

            # ---------------- passes 1..NPASS: T steps each ------------------
            # SBUF strip layout: contiguous row blocks. Tile rows [0, RHs) on
            # partitions 0:64 (block A), rows [RHs, Rs) on 64:128 (block B).
            # All of an interior row's taps source its own block, so each
            # PSUM accumulation group uses a single tile_position row-group
            # (mixed row-groups in one group crash the device). The two seam
            # rows (RHs-1, RHs) put their <=3 cross-block taps into a separate
            # single-source PSUM group, folded in with one extra DVE add.
            # Taps are emitted round-robin across the 4 in-flight rows so the
            # in-order PE FIFO interleaves all 4 quadrant streams.
            with tc.tile_pool(name="xs", bufs=2) as xpool, \
                 tc.tile_pool(name="chs", bufs=2) as chpool, \
                 tc.tile_pool(name="th", bufs=g["RH"] // 4 + 1) as thpool, \
                 tc.tile_pool(name="ps", bufs=3, space="PSUM") as pspool, \
                 tc.tile_pool(name="pf", bufs=2, space="PSUM") as pfpool:
                def strip_load(p, o0, hs):
                    src_d = Xd[(p - 1) % 2]
                    Rs = hs + 2 * T + 2
                    RHs = Rs // 2
                    base = o0 - (T + 1)              # slab row of tile row 0
                    sv_lo = max(0, -base)
                    sv_hi = min(Rs, SLAB - base)
                    xs = xpool.tile([128, RHs, WP], F16, tag="xs")
                    ch = chpool.tile([128, RHs, WP], F16, tag="ch")
                    for blk in range(2):
                        lo, hi = blk * RHs, (blk + 1) * RHs
                        pr = slice(blk * 64, blk * 64 + 64)
                        ld_lo, ld_hi = max(lo, sv_lo), min(hi, sv_hi)
                        if ld_lo > lo:
                            nc.vector.memset(xs[pr, 0:ld_lo - lo, :], 0.0)
                        if ld_hi < hi:
                            nc.vector.memset(xs[pr, ld_hi - lo:RHs, :], 0.0)
                        if p > 1:
                            nc.sync.dma_start(
                                out=xs[pr, ld_lo - lo:ld_hi - lo, :],
                                in_=src_d[:, base + ld_lo:base + ld_hi, :])
                        nc.sync.dma_start(
                            out=ch[pr, ld_lo - lo:ld_hi - lo, :],
                            in_=Chi_d[:, base + ld_lo:base + ld_hi, :])
                    def derive():
                        # x0 = control = C/beta - bias, derived from C in
                        # 4-row chunks alternated across Scalar and DVE,
                        # restricted to rows whose C was actually loaded.
                        # Deferred into the previous strip's last step so it
                        # never head-of-line blocks the scalar queue while
                        # the ch DMA is still in flight.
                        nc.vector.memset(xs[:, :, 0:1], 0.0)
                        nc.vector.memset(xs[:, :, 513:514], 0.0)
                        a = (max(0, sv_lo), min(RHs, sv_hi))
                        b = (max(0, sv_lo - RHs), min(RHs, sv_hi - RHs))
                        lo2, hi2 = max(a[0], b[0]), min(a[1], b[1])
                        segs = []
                        if lo2 < hi2:
                            segs.append((slice(0, 128), lo2, hi2))
                        for (prg, v) in ((slice(0, 64), a),
                                         (slice(64, 128), b)):
                            if v[0] < min(v[1], lo2):
                                segs.append((prg, v[0], min(v[1], lo2)))
                            if max(v[0], hi2) < v[1]:
                                segs.append((prg, max(v[0], hi2), v[1]))
                        cnt = 0
                        for (prg, lo_, hi_) in segs:
                            for j in range(lo_, hi_, 4):
                                n = min(4, hi_ - j)
                                cnt += 1
                                if cnt % 2:
                                    nc.scalar.activation(
                                        out=xs[prg, j:j + n, 1:513],
                                        in_=ch[prg, j:j + n, 1:513],
                                        func=AF.Identity,
                                        scale=invb_t[prg],
                                        bias=mbias_t[prg])
                                else:
                                    nc.vector.tensor_scalar(
                                        out=xs[prg, j:j + n, 1:513],
                                        in0=ch[prg, j:j + n, 1:513],
                                        scalar1=invb_t[prg], op0=ALU.mult,
                                        scalar2=mbias_t[prg], op1=ALU.add)
                    return xs, ch, (derive if p == 1 else None)

                def mk_tanh(xs_, RHs_):
                    def emit(t):
                        j = 4 * t
                        n = min(4, RHs_ - j)
                        tt = thpool.tile([128, 4, WP], F16)
                        nc.scalar.activation(out=tt[:, 0:n, :],
                                             in_=xs_[:, j:j + n, :],
                                             func=AF.Tanh)
                        return tt
                    return emit

                def strip_compute(p, o0, hs, xs, ch, thp1, nxt,
                                  nxt_finish=None):
                    dst_d = Xd[p % 2]
                    last = (p == NPASS)
                    Rs = hs + 2 * T + 2
                    RHs = Rs // 2
                    base = o0 - (T + 1)
                    sv_lo = max(0, -base)
                    sv_hi = min(Rs, SLAB - base)
                    NT = (RHs + 3) // 4
                    emit_tanh_tile = mk_tanh(xs, RHs)
                    # next strip's step-1 tanh tiles, emitted during this
                    # strip's last step so the next strip starts hot
                    if nxt is not None:
                        nxt_emit = mk_tanh(nxt[0], nxt[1])
                        nxt_NT = (nxt[1] + 3) // 4
                    else:
                        nxt_NT = 0
                    nxt_thp = []

                    # tanh tiles for step 1; later steps' tanhs are emitted
                    # inside the previous step's group loop as soon as their
                    # rows are evacuated, so the scalar engine works ahead.
                    thp_next = (thp1 if thp1 is not None
                                else [emit_tanh_tile(t) for t in range(NT)])
                    if True:
                        for k in range(1, T + 1):
                            # fixed window: rows outside [k, Rs-k) are never
                            # stored or read by later exact rows, so compute
                            # them anyway -- keeps every group 4-stream wide.
                            up_lo = max(1, sv_lo)
                            up_hi = min(Rs - 1, sv_hi)
                            thp = thp_next
                            thp_next = []


                            def th_row(sj):
                                return thp[sj // 4][:, sj % 4, :]

                            def row_taps(trow, ph, ps_tile, pf_tile,
                                         add_c=False):
                                """Build this row's matmul arg-list (main
                                group, then foreign group). Returns
                                (list of matmul kwargs, used_foreign)."""
                                dblk, dj = divmod(trow, RHs)
                                dp = slice(dblk * 64, dblk * 64 + 64)
                                out_ps = ps_tile[ph * 64:ph * 64 + 64, :]
                                main, foreign = [], []
                                for t9 in range(9):
                                    kh, kw = divmod(t9, 3)
                                    srow = trow + kh - 1
                                    sblk, sj = divmod(srow, RHs)
                                    (main if sblk == dblk else foreign).append(
                                        (t9, sblk, sj, kw))
                                ops = []
                                for i, (t9, sblk, sj, kw) in enumerate(main):
                                    ops.append(dict(
                                        out=out_ps, lhsT=wa_t[dp, t9, :],
                                        rhs=th_row(sj)[dp, kw:kw + 512],
                                        start=(i == 0), stop=False,
                                        tile_position=(dblk * 64, ph * 64)))
                                ops.append(dict(
                                    out=out_ps, lhsT=wa_t[dp, 9, :],
                                    rhs=xs[dp, dj, 1:513],
                                    start=False, stop=not add_c,
                                    tile_position=(dblk * 64, ph * 64)))
                                if add_c:
                                    # fold C into PSUM so this stream's evac
                                    # is a single scalar copy (no gpsimd hop
                                    # on the tanh critical path)
                                    ops.append(dict(
                                        out=out_ps, lhsT=wa_t[dp, 10, :],
                                        rhs=ch[dp, dj, 1:513],
                                        start=False, stop=True,
                                        tile_position=(dblk * 64, ph * 64)))
                                if foreign:
                                    sblk = foreign[0][1]
                                    sp = slice(sblk * 64, sblk * 64 + 64)
                                    out_pf = pf_tile[ph * 64:ph * 64 + 64, :]
                                    for i, (t9, _, sj, kw) in enumerate(foreign):
                                        ops.append(dict(
                                            out=out_pf, lhsT=wa_t[sp, t9, :],
                                            rhs=th_row(sj)[sp, kw:kw + 512],
                                            start=(i == 0),
                                            stop=(i == len(foreign) - 1),
                                            tile_position=(sblk * 64,
                                                           ph * 64)))
                                return ops, bool(foreign)

                            def fadd(prow, prange, in1):
                                nc.vector.scalar_tensor_tensor(
                                    out=xs[prange, prow, 1:513],
                                    in0=xs[prange, prow, 1:513],
                                    scalar=1.0, in1=in1,
                                    op0=ALU.bypass, op1=ALU.add)

                            def cadd(prow, prange):
                                nc.gpsimd.tensor_tensor(
                                    out=xs[prange, prow, 1:513],
                                    in0=xs[prange, prow, 1:513],
                                    in1=ch[prange, prow, 1:513],
                                    op=ALU.add)

                            def act(trow):
                                return up_lo <= trow < up_hi

                            def fstt(prow, prange, in1):
                                nc.vector.scalar_tensor_tensor(
                                    out=xs[prange, prow, 1:513],
                                    in0=in1, scalar=1.0,
                                    in1=ch[prange, prow, 1:513],
                                    op0=ALU.bypass, op1=ALU.add)

                            def build_pair(j0):
                                j1 = j0 + 1
                                a0, b0 = act(j0), act(RHs + j0)
                                if j1 < RHs:
                                    a1, b1 = act(j1), act(RHs + j1)
                                else:
                                    a1 = b1 = False
                                P0 = P1 = PF = None
                                need_pf = ((a0 and j0 == RHs - 1)
                                           or (a1 and j1 == RHs - 1)
                                           or (b0 and j0 == 0))
                                if need_pf:
                                    PF = pfpool.tile([128, 512], F32)
                                if a0 or b0:
                                    P0 = pspool.tile([128, 512], F32, tag="P0")
                                if a1 or b1:
                                    P1 = pspool.tile([128, 512], F32, tag="P1")
                                seqs = []
                                frows = []
                                if a0:
                                    s, f = row_taps(j0, 0, P0, PF)
                                    seqs.append(s)
                                    if f:
                                        frows.append((j0, slice(0, 64), 0))
                                if b0:
                                    s, f = row_taps(RHs + j0, 1, P0, PF)
                                    seqs.append(s)
                                    if f:
                                        frows.append((j0, slice(64, 128), 1))
                                if a1:
                                    s, f = row_taps(j1, 1, P1, PF)
                                    seqs.append(s)
                                    if f:
                                        frows.append((j1, slice(0, 64), 1))
                                if b1:
                                    s, f = row_taps(RHs + j1, 0, P1, PF,
                                                    add_c=True)
                                    seqs.append(s)
                                    if f:
                                        frows.append((j1, slice(64, 128), 0))
                                return dict(j0=j0, j1=j1, a0=a0, b0=b0,
                                            a1=a1, b1=b1, P0=P0, P1=P1,
                                            PF=PF, seqs=seqs, frows=frows)

                            def evac_pair(pr):
                                # x' = psum + C (alpha*x folded into tap 9):
                                # j0 + j1-A fused stt on DVE (psum + ch);
                                # j1-B Scalar copy then gpsimd SBUF add.
                                j0, j1 = pr["j0"], pr["j1"]
                                P0, P1, PF = pr["P0"], pr["P1"], pr["PF"]
                                if pr["a0"] and pr["b0"]:
                                    fstt(j0, slice(0, 128), P0)
                                elif pr["a0"]:
                                    fstt(j0, slice(0, 64), P0[0:64, :])
                                elif pr["b0"]:
                                    fstt(j0, slice(64, 128), P0[64:128, :])
                                if pr["a1"]:
                                    fstt(j1, slice(0, 64), P1[64:128, :])
                                if pr["b1"]:
                                    nc.scalar.activation(
                                        out=xs[64:128, j1, 1:513],
                                        in_=P1[0:64, :],
                                        func=AF.Copy, scale=1.0)
                                for (pj, xsl, phh) in pr["frows"]:
                                    fadd(pj, xsl,
                                         PF[phh * 64:phh * 64 + 64, :])

                            # round-robin taps across the 4 streams of each
                            # row-pair -> 4 concurrent PE quadrant streams.
                            for j0 in range(0, RHs, 2):
                                pr = build_pair(j0)
                                nmax = max((len(s) for s in pr["seqs"]),
                                           default=0)
                                for t in range(nmax):
                                    for s in pr["seqs"]:
                                        if t < len(s):
                                            nc.tensor.matmul(
                                                s[t]["out"], s[t]["lhsT"],
                                                s[t]["rhs"],
                                                start=s[t]["start"],
                                                stop=s[t]["stop"],
                                                skip_group_check=True,
                                                tile_position=s[t][
                                                    "tile_position"])
                                evac_pair(pr)
                                if k < T:
                                    while (len(thp_next) < NT
                                           and 4 * len(thp_next) + 3
                                           <= j0 + 1):
                                        thp_next.append(
                                            emit_tanh_tile(len(thp_next)))
                                elif (j0 % 4 == 2
                                      and len(nxt_thp) < nxt_NT):
                                    nxt_thp.append(
                                        nxt_emit(len(nxt_thp)))
                            if k < T:
                                while len(thp_next) < NT:
                                    thp_next.append(
                                        emit_tanh_tile(len(thp_next)))
                            else:
                                while len(nxt_thp) < nxt_NT:
                                    nxt_thp.append(nxt_emit(len(nxt_thp)))
                        # store valid rows (tile rows [T+1, Rs-T-1))
                        st_lo, st_hi = T + 1, Rs - (T + 1)
                        for blk in range(2):
                            lo, hi = blk * RHs, (blk + 1) * RHs
                            pr = slice(blk * 64, blk * 64 + 64)
                            s_lo, s_hi = max(lo, st_lo), min(hi, st_hi)
                            if s_lo >= s_hi:
                                continue
                            dst = x_out if last else dst_d
                            nc.sync.dma_start(
                                out=dst[:, base + s_lo:base + s_hi, :],
                                in_=xs[pr, s_lo - lo:s_hi - lo, :])
                    return nxt_thp

                # software pipeline: strip i+1's loads are emitted (and hence
                # enqueued on the sync/DMA engines) before strip i's compute,
                # so DMA overlaps compute via the bufs=2 tile rings.
                items = [(p, o0, hs) for p in range(1, NPASS + 1)
                         for (o0, hs) in g["STRIPS"]]
                def do_load(it):
                    r = strip_load(*it)
                    if r[2] is not None:
                        r[2]()
                    return r
                cur = do_load(items[0])
                thp1 = None
                for i, it in enumerate(items):
                    if i + 1 < len(items):
                        nxt_item = items[i + 1]
                        nxt_tiles = do_load(nxt_item)
                        nxt_RHs = (nxt_item[2] + 2 * T + 2) // 2
                        nxt = (nxt_tiles[0], nxt_RHs)
                    else:
                        nxt_tiles = nxt = None
                    thp1 = strip_compute(*it, cur[0], cur[1], thp1, nxt)
                    cur = nxt_tiles

    nc.compile()
    return nc


def host_prep(u, W_B, W_A, bias, alpha_logit, cfg):
    """Build per-core input maps. Only valid for the full-size problem."""
    g = _derive(cfg)
    SLAB, UROWS = g["SLAB"], g["UROWS"]
    B = u.shape[0]
    H = u.shape[2]
    Wc = 512

    alpha = np.float32(1.0 / (1.0 + np.exp(-np.float64(alpha_logit))))
    beta = np.float32(1.0) - alpha

    WAe = np.array(W_A, dtype=np.float32).copy()
    idx = np.arange(64)
    WAe[idx, idx, 1, 1] = np.maximum(WAe[idx, idx, 1, 1], np.float32(1.0))

    wa_taps = np.zeros((64, 11, 64), dtype=np.float32)
    for t9 in range(9):
        kh, kw = divmod(t9, 3)
        wa_taps[:, t9, :] = (beta * WAe[:, :, kh, kw]).T   # [cin, cout]
    wa_taps[:, 9, :] = alpha * np.eye(64, dtype=np.float32)
    wa_taps[:, 10, :] = np.eye(64, dtype=np.float32)
    wa_taps = wa_taps.astype(np.float16)

    bias_vec = np.array(bias, dtype=np.float32).reshape(64)
    wb9 = np.zeros((9, 64), dtype=np.float32)
    for kw in range(3):
        for kh in range(3):
            wb9[kw * 3 + kh, :] = W_B[:, 0, kh, kw]
    wb_hi = wb9.astype(ml_dtypes.bfloat16)
    wb_lo = (wb9 - wb_hi.astype(np.float32)).astype(ml_dtypes.bfloat16)
    wb27 = np.concatenate([wb_hi, wb_hi, wb_lo], axis=0)
    nbias = (beta * bias_vec).reshape(64, 1).astype(np.float32)
    mb = (-bias_vec).reshape(64, 1).astype(np.float32)
    alpha_arr = np.full((1, 1), alpha, dtype=np.float32)

    in_maps = []
    for core in range(8):
        b, h = divmod(core, 2)
        img = np.asarray(u[b, 0], dtype=np.float32)        # [H, 512]
        u_slab = np.zeros((UROWS, Wc), dtype=np.float32)
        if h == 0:
            # slab rows [-1, SLAB+1) = image rows [-1, SLAB+1)
            u_slab[1:UROWS] = img[0:SLAB + 1]
        else:
            off = H - SLAB                                  # 240
            # slab row s = image row s + off; u_in[j] = image j-1+off
            u_slab[0:UROWS - 1] = img[off - 1:H]
        u_hi = u_slab.astype(ml_dtypes.bfloat16)
        u_lo = (u_slab - u_hi.astype(np.float32)).astype(ml_dtypes.bfloat16)
        # im2col: u27[third*9 + kw*3 + kh, j, c] = src[j + kh, c + kw - 1]
        # (src rows are already the +-1 padded slab; cols zero-padded)
        u27 = np.zeros((27, SLAB, Wc), dtype=ml_dtypes.bfloat16)
        for third, src in enumerate((u_hi, u_lo, u_hi)):
            for kw in range(3):
                c_lo = max(0, 1 - kw)
                c_hi = min(Wc, Wc + 1 - kw)
                for kh in range(3):
                    t27 = third * 9 + kw * 3 + kh
                    u27[t27, :, c_lo:c_hi] = src[kh:kh + SLAB,
                                                 c_lo + kw - 1:c_hi + kw - 1]
        in_maps.append({
            "u27_in": u27,
            "wa_in": wa_taps,
            "wb_in": wb27,
            "nbias_in": nbias,
            "mb_in": mb,
            "alpha_in": alpha_arr,
        })
    return in_maps


_NC_CACHE = {}


def _get_nc(cfg_key=None):
    if "nc" not in _NC_CACHE:
        _NC_CACHE["nc"] = build(FULL_CFG)
    return _NC_CACHE["nc"]


def kernel(u, W_B, W_A, bias, alpha_logit, _trace=False):
    u = np.asarray(u, dtype=np.float32)
    B, _, H, Wc = u.shape
    nc = _get_nc()
    in_maps = host_prep(u, W_B, W_A, bias, alpha_logit, FULL_CFG)
    res = run_bass_kernel_spmd(nc, in_maps, core_ids=list(range(8)),
                               trace=_trace)
    SLAB = FULL_CFG["SLAB"]
    VALID = H // 2                                          # 256
    out = np.zeros((B, 64, H, Wc), dtype=np.float32)
    for core in range(8):
        b, h = divmod(core, 2)
        xo = res.results[core]["x_out"]                     # [64, SLAB, 514] f16
        xo = np.asarray(xo[:, :, 1:513], dtype=np.float32)
        if h == 0:
            out[b, :, 0:VALID, :] = xo[:, 0:VALID, :]
        else:
            out[b, :, VALID:H, :] = xo[:, SLAB - VALID:SLAB, :]
    kernel._last_results = res
    return out


# revision 60
# speedup vs baseline: 1.0215x; 1.0215x over previous
"""CeNN front-end Trainium2 kernel.

Reference computation (per batch image u [1,H,W]):
    control = conv3x3_same(u, W_B) + 0                     # [64,H,W]
    x0 = control
    x_{k+1} = alpha*x_k + beta*(conv3x3_same(tanh(x_k), WA_eff) + control + bias)
    (WA_eff = W_A with diagonal center taps clamped >= 1), 16 steps.

Distribution: 8 cores = (batch b in 0..3) x (H half). Each core owns a
272-row slab (256 valid + 16 halo rows toward the other half). Zero
communication: halo contamination advances one row per step and after 16
steps exactly the 16 halo rows are dirty.

Per-core kernel: channel-major layout [64ch -> partitions, rows, 514 cols
(W+2 zero pad)]. Rows are split into two 64-partition blocks (A on
partitions 0:64, B on 64:128) so elementwise work runs 128 wide.

conv3x3 = 9 accumulating matmuls (K=64 cin, M=64 cout, N=512) at per-tap
free offsets, in bf16 (tanh output + beta-scaled weights).  One more
identity tap adds C = beta*(control+bias) as a single bf16 term.  Four PE
quadrants (tile_position) process four rows concurrently.  The state
update is one fused DVE op per row:
    x' = (x * alpha) + psum        (scalar_tensor_tensor)

Time is blocked T=4 steps per pass over fp16 DRAM ping-pong buffers with
redundant-halo strips.  Pass 0 computes control from u with a K=10 fp32
im2col matmul (9 shifted u copies + ones row for bias).  x_out is fp16,
514-wide padded for contiguous stores; host slices + converts.
"""

import math

import numpy as np
import ml_dtypes

import concourse.bacc as bacc
import concourse.tile as tile
from concourse import mybir
from concourse.bass_utils import run_bass_kernel_spmd

F32 = mybir.dt.float32
F32R = mybir.dt.float32r
F16 = mybir.dt.float16
BF16 = mybir.dt.bfloat16
AF = mybir.ActivationFunctionType
ALU = mybir.AluOpType

FULL_CFG = dict(SLAB=272, HS=62, T=4, NSTEPS=16, RC0=34)


def _derive(cfg):
    d = dict(cfg)
    d["R"] = d["HS"] + 2 * d["T"] + 2          # strip tile rows
    assert d["R"] % 2 == 0
    d["RH"] = d["R"] // 2                      # rows per partition block
    # strips may be ragged: last strip covers the remainder
    strips = []
    o0 = 0
    while o0 < d["SLAB"]:
        hs = min(d["HS"], d["SLAB"] - o0)
        assert hs % 2 == 0
        strips.append((o0, hs))
        o0 += hs
    d["STRIPS"] = strips
    d["NSTRIP"] = len(strips)
    assert d["SLAB"] % d["RC0"] == 0
    d["NCHUNK0"] = d["SLAB"] // d["RC0"]
    assert d["NSTEPS"] % d["T"] == 0
    d["NPASS"] = d["NSTEPS"] // d["T"]
    d.setdefault("DBG_P0_OUT", d["NPASS"] == 0)
    d["UROWS"] = d["SLAB"] + 2
    return d


def build(cfg):
    """Build the per-core Bass program. Returns compiled nc."""
    g = _derive(cfg)
    SLAB, HS, T, RC0 = g["SLAB"], g["HS"], g["T"], g["RC0"]
    R, RH, NSTRIP, NCHUNK0, NPASS, UROWS = (
        g["R"], g["RH"], g["NSTRIP"], g["NCHUNK0"], g["NPASS"], g["UROWS"])
    WP = 514
    W = 512

    nc = bacc.Bacc("TRN2", target_bir_lowering=False, debug=False,
                   num_devices=8)

    u27_in = nc.dram_tensor("u27_in", [27, SLAB, W], BF16,
                            kind="ExternalInput")
    wa_in = nc.dram_tensor("wa_in", [64, 11, 64], F16, kind="ExternalInput")
    wb_in = nc.dram_tensor("wb_in", [27, 64], BF16, kind="ExternalInput")
    nbias_in = nc.dram_tensor("nbias_in", [64, 1], F32, kind="ExternalInput")
    mb_in = nc.dram_tensor("mb_in", [64, 1], F32, kind="ExternalInput")
    alpha_in = nc.dram_tensor("alpha_in", [1, 1], F32, kind="ExternalInput")
    x_out = nc.dram_tensor("x_out", [64, SLAB, WP], F16, kind="ExternalOutput")

    Xd = [nc.dram_tensor(f"Xd{i}", [64, SLAB, WP], F16, kind="Internal")
          for i in range(2)]
    Chi_d = nc.dram_tensor("Chi", [64, SLAB, WP], F16, kind="Internal")

    with tile.TileContext(nc) as tc:
        with tc.tile_pool(name="singles", bufs=1) as singles:
            wa_t = singles.tile([128, 11, 64], F16)
            nc.sync.dma_start(out=wa_t[0:64], in_=wa_in[:, :, :])
            nc.sync.dma_start(out=wa_t[64:128], in_=wa_in[:, :, :])
            wb_t = singles.tile([27, 64], BF16)
            nc.sync.dma_start(out=wb_t, in_=wb_in[:, :])
            nbias_t = singles.tile([128, 1], F32)
            nc.sync.dma_start(out=nbias_t[0:64], in_=nbias_in[:, :])
            nc.sync.dma_start(out=nbias_t[64:128], in_=nbias_in[:, :])
            mbias_t = singles.tile([128, 1], F32)
            nc.sync.dma_start(out=mbias_t[0:64], in_=mb_in[:, :])
            nc.sync.dma_start(out=mbias_t[64:128], in_=mb_in[:, :])
            alpha_t = singles.tile([128, 1], F32)
            nc.sync.dma_start(out=alpha_t, in_=alpha_in[:, :].to_broadcast((128, 1)))
            beta_t = singles.tile([128, 1], F32)
            nc.vector.tensor_scalar(out=beta_t, in0=alpha_t, scalar1=-1.0,
                                    scalar2=1.0, op0=ALU.mult, op1=ALU.add)
            invb_t = singles.tile([128, 1], F32)
            nc.vector.reciprocal(out=invb_t, in_=beta_t)

            # ---------------- pass 0: control -> C only ----------------------
            # One K=27 bf16 matmul per row (u split hi+lo on host, weights
            # hi+lo: uh*wh + ul*wh + uh*wl), two rows concurrent via the two
            # PE column groups.  Only C = beta*(control+bias) is stored;
            # pass 1 derives x0 = C/beta - bias from the loaded C tile.
            with tc.tile_pool(name="p0u", bufs=3) as p0u, \
                 tc.tile_pool(name="p0ps", bufs=4, space="PSUM") as p0ps, \
                 tc.tile_pool(name="p0st", bufs=3) as p0st:
                def p0_load(chk):
                    # host prebuilds the 27-tap im2col (hi/lo split + kh/kw
                    # shifts), so each chunk is a single contiguous DMA --
                    # the sync engine's ~1us per-dma_start issue cost made
                    # 27 small gathers the pass-0 rate limiter.
                    c0 = RC0 * chk
                    u9 = p0u.tile([27, RC0, W], BF16)
                    nc.sync.dma_start(out=u9, in_=u27_in[:, c0:c0 + RC0, :])
                    return u9

                u9s = [p0_load(0), p0_load(1)]
                for chk in range(NCHUNK0):
                    c0 = RC0 * chk
                    if chk + 2 < NCHUNK0:
                        u9s.append(p0_load(chk + 2))
                    u9 = u9s[chk]
                    chst = p0st.tile([64, RC0, WP], F16, tag="chst")
                    nc.vector.memset(chst[:, :, 0:1], 0.0)
                    nc.vector.memset(chst[:, :, 513:514], 0.0)
                    for t in range(0, RC0, 2):
                        pc = p0ps.tile([128, 512], F32)
                        nc.tensor.matmul(pc[0:64, :], wb_t, u9[:, t, :],
                                         start=True, stop=True,
                                         tile_position=(0, 0))
                        nc.tensor.matmul(pc[64:128, :], wb_t, u9[:, t + 1, :],
                                         start=True, stop=True,
                                         skip_group_check=True,
                                         tile_position=(0, 64))
                        # C = beta*psum + beta*bias, split Scalar/DVE
                        nc.scalar.activation(out=chst[:, t, 1:513],
                                             in_=pc[0:64, :],
                                             func=AF.Identity,
                                             scale=beta_t[0:64],
                                             bias=nbias_t[0:64])
                        nc.vector.tensor_scalar(
                            out=chst[:, t + 1, 1:513], in0=pc[64:128, :],
                            scalar1=beta_t[64:128], op0=ALU.mult,
                            scalar2=nbias_t[64:128], op1=ALU.add)
                    nc.sync.dma_start(out=Chi_d[:, c0:c0 + RC0, :],
                                        in_=chst)

            # ---------------- passes 1..NPASS: T steps each ------------------
            # SBUF strip layout: contiguous row blocks. Tile rows [0, RHs) on
            # partitions 0:64 (block A), rows [RHs, Rs) on 64:128 (block B).
            # All of an interior row's taps source its own block, so each
            # PSUM accumulation group uses a single tile_position row-group
            # (mixed row-groups in one group crash the device). The two seam
            # rows (RHs-1, RHs) put their <=3 cross-block taps into a separate
            # single-source PSUM group, folded in with one extra DVE add.
            # Taps are emitted round-robin across the 4 in-flight rows so the
            # in-order PE FIFO interleaves all 4 quadrant streams.
            with tc.tile_pool(name="xs", bufs=2) as xpool, \
                 tc.tile_pool(name="chs", bufs=2) as chpool, \
                 tc.tile_pool(name="th", bufs=g["RH"] // 4 + 3) as thpool, \
                 tc.tile_pool(name="ps", bufs=3, space="PSUM") as pspool, \
                 tc.tile_pool(name="pf", bufs=2, space="PSUM") as pfpool:
                def strip_load(p, o0, hs):
                    src_d = Xd[(p - 1) % 2]
                    Rs = hs + 2 * T + 2
                    RHs = Rs // 2
                    base = o0 - (T + 1)              # slab row of tile row 0
                    sv_lo = max(0, -base)
                    sv_hi = min(Rs, SLAB - base)
                    xs = xpool.tile([128, RHs, WP], F16, tag="xs")
                    ch = chpool.tile([128, RHs, WP], F16, tag="ch")
                    for blk in range(2):
                        lo, hi = blk * RHs, (blk + 1) * RHs
                        pr = slice(blk * 64, blk * 64 + 64)
                        ld_lo, ld_hi = max(lo, sv_lo), min(hi, sv_hi)
                        if ld_lo > lo:
                            nc.vector.memset(xs[pr, 0:ld_lo - lo, :], 0.0)
                        if ld_hi < hi:
                            nc.vector.memset(xs[pr, ld_hi - lo:RHs, :], 0.0)
                        if p > 1:
                            nc.sync.dma_start(
                                out=xs[pr, ld_lo - lo:ld_hi - lo, :],
                                in_=src_d[:, base + ld_lo:base + ld_hi, :])
                        nc.sync.dma_start(
                            out=ch[pr, ld_lo - lo:ld_hi - lo, :],
                            in_=Chi_d[:, base + ld_lo:base + ld_hi, :])
                    def derive():
                        # x0 = control = C/beta - bias, derived from C in
                        # 4-row chunks alternated across Scalar and DVE,
                        # restricted to rows whose C was actually loaded.
                        # Deferred into the previous strip's last step so it
                        # never head-of-line blocks the scalar queue while
                        # the ch DMA is still in flight.
                        nc.vector.memset(xs[:, :, 0:1], 0.0)
                        nc.vector.memset(xs[:, :, 513:514], 0.0)
                        a = (max(0, sv_lo), min(RHs, sv_hi))
                        b = (max(0, sv_lo - RHs), min(RHs, sv_hi - RHs))
                        lo2, hi2 = max(a[0], b[0]), min(a[1], b[1])
                        segs = []
                        if lo2 < hi2:
                            segs.append((slice(0, 128), lo2, hi2))
                        for (prg, v) in ((slice(0, 64), a),
                                         (slice(64, 128), b)):
                            if v[0] < min(v[1], lo2):
                                segs.append((prg, v[0], min(v[1], lo2)))
                            if max(v[0], hi2) < v[1]:
                                segs.append((prg, max(v[0], hi2), v[1]))
                        cnt = 0
                        for (prg, lo_, hi_) in segs:
                            for j in range(lo_, hi_, 4):
                                n = min(4, hi_ - j)
                                cnt += 1
                                if cnt % 2:
                                    nc.scalar.activation(
                                        out=xs[prg, j:j + n, 1:513],
                                        in_=ch[prg, j:j + n, 1:513],
                                        func=AF.Identity,
                                        scale=invb_t[prg],
                                        bias=mbias_t[prg])
                                else:
                                    nc.vector.tensor_scalar(
                                        out=xs[prg, j:j + n, 1:513],
                                        in0=ch[prg, j:j + n, 1:513],
                                        scalar1=invb_t[prg], op0=ALU.mult,
                                        scalar2=mbias_t[prg], op1=ALU.add)
                    return xs, ch, (derive if p == 1 else None)

                def mk_tanh(xs_, RHs_):
                    def emit(t):
                        j = 4 * t
                        n = min(4, RHs_ - j)
                        tt = thpool.tile([128, 4, WP], F16)
                        nc.scalar.activation(out=tt[:, 0:n, :],
                                             in_=xs_[:, j:j + n, :],
                                             func=AF.Tanh)
                        return tt
                    return emit

                def strip_compute(p, o0, hs, xs, ch, thp1, nxt,
                                  nxt_finish=None):
                    dst_d = Xd[p % 2]
                    last = (p == NPASS)
                    Rs = hs + 2 * T + 2
                    RHs = Rs // 2
                    base = o0 - (T + 1)
                    sv_lo = max(0, -base)
                    sv_hi = min(Rs, SLAB - base)
                    NT = (RHs + 3) // 4
                    emit_tanh_tile = mk_tanh(xs, RHs)
                    # next strip's step-1 tanh tiles, emitted during this
                    # strip's last step so the next strip starts hot
                    if nxt is not None:
                        nxt_emit = mk_tanh(nxt[0], nxt[1])
                        nxt_NT = (nxt[1] + 3) // 4
                    else:
                        nxt_NT = 0
                    nxt_thp = []

                    # tanh tiles for step 1; later steps' tanhs are emitted
                    # inside the previous step's group loop as soon as their
                    # rows are evacuated, so the scalar engine works ahead.
                    thp_next = (thp1 if thp1 is not None
                                else [emit_tanh_tile(t) for t in range(NT)])
                    if True:
                        for k in range(1, T + 1):
                            # fixed window: rows outside [k, Rs-k) are never
                            # stored or read by later exact rows, so compute
                            # them anyway -- keeps every group 4-stream wide.
                            up_lo = max(1, sv_lo)
                            up_hi = min(Rs - 1, sv_hi)
                            thp = thp_next
                            thp_next = []


                            def th_row(sj):
                                return thp[sj // 4][:, sj % 4, :]

                            def row_taps(trow, ph, ps_tile, pf_tile,
                                         add_c=False):
                                """Build this row's matmul arg-list (main
                                group, then foreign group). Returns
                                (list of matmul kwargs, used_foreign)."""
                                dblk, dj = divmod(trow, RHs)
                                dp = slice(dblk * 64, dblk * 64 + 64)
                                out_ps = ps_tile[ph * 64:ph * 64 + 64, :]
                                main, foreign = [], []
                                for t9 in range(9):
                                    kh, kw = divmod(t9, 3)
                                    srow = trow + kh - 1
                                    sblk, sj = divmod(srow, RHs)
                                    (main if sblk == dblk else foreign).append(
                                        (t9, sblk, sj, kw))
                                ops = []
                                for i, (t9, sblk, sj, kw) in enumerate(main):
                                    ops.append(dict(
                                        out=out_ps, lhsT=wa_t[dp, t9, :],
                                        rhs=th_row(sj)[dp, kw:kw + 512],
                                        start=(i == 0), stop=False,
                                        tile_position=(dblk * 64, ph * 64)))
                                ops.append(dict(
                                    out=out_ps, lhsT=wa_t[dp, 9, :],
                                    rhs=xs[dp, dj, 1:513],
                                    start=False, stop=not add_c,
                                    tile_position=(dblk * 64, ph * 64)))
                                if add_c:
                                    # fold C into PSUM so this stream's evac
                                    # is a single scalar copy (no gpsimd hop
                                    # on the tanh critical path)
                                    ops.append(dict(
                                        out=out_ps, lhsT=wa_t[dp, 10, :],
                                        rhs=ch[dp, dj, 1:513],
                                        start=False, stop=True,
                                        tile_position=(dblk * 64, ph * 64)))
                                if foreign:
                                    sblk = foreign[0][1]
                                    sp = slice(sblk * 64, sblk * 64 + 64)
                                    out_pf = pf_tile[ph * 64:ph * 64 + 64, :]
                                    for i, (t9, _, sj, kw) in enumerate(foreign):
                                        ops.append(dict(
                                            out=out_pf, lhsT=wa_t[sp, t9, :],
                                            rhs=th_row(sj)[sp, kw:kw + 512],
                                            start=(i == 0),
                                            stop=(i == len(foreign) - 1),
                                            tile_position=(sblk * 64,
                                                           ph * 64)))
                                return ops, bool(foreign)

                            def fadd(prow, prange, in1):
                                nc.vector.scalar_tensor_tensor(
                                    out=xs[prange, prow, 1:513],
                                    in0=xs[prange, prow, 1:513],
                                    scalar=1.0, in1=in1,
                                    op0=ALU.bypass, op1=ALU.add)

                            def cadd(prow, prange):
                                nc.gpsimd.tensor_tensor(
                                    out=xs[prange, prow, 1:513],
                                    in0=xs[prange, prow, 1:513],
                                    in1=ch[prange, prow, 1:513],
                                    op=ALU.add)

                            def act(trow):
                                return up_lo <= trow < up_hi

                            def fstt(prow, prange, in1):
                                nc.vector.scalar_tensor_tensor(
                                    out=xs[prange, prow, 1:513],
                                    in0=in1, scalar=1.0,
                                    in1=ch[prange, prow, 1:513],
                                    op0=ALU.bypass, op1=ALU.add)

                            def build_pair(j0):
                                codd = (j0 // 2) % 2 == 1
                                j1 = j0 + 1
                                a0, b0 = act(j0), act(RHs + j0)
                                if j1 < RHs:
                                    a1, b1 = act(j1), act(RHs + j1)
                                else:
                                    a1 = b1 = False
                                P0 = P1 = PF = None
                                need_pf = ((a0 and j0 == RHs - 1)
                                           or (a1 and j1 == RHs - 1)
                                           or (b0 and j0 == 0))
                                if need_pf:
                                    PF = pfpool.tile([128, 512], F32)
                                if a0 or b0:
                                    P0 = pspool.tile([128, 512], F32, tag="P0")
                                if a1 or b1:
                                    P1 = pspool.tile([128, 512], F32, tag="P1")
                                seqs = []
                                frows = []
                                if a0:
                                    s, f = row_taps(j0, 0, P0, PF)
                                    seqs.append(s)
                                    if f:
                                        frows.append((j0, slice(0, 64), 0))
                                if b0:
                                    s, f = row_taps(RHs + j0, 1, P0, PF)
                                    seqs.append(s)
                                    if f:
                                        frows.append((j0, slice(64, 128), 1))
                                if a1:
                                    s, f = row_taps(j1, 1, P1, PF,
                                                    add_c=codd)
                                    seqs.append(s)
                                    if f:
                                        frows.append((j1, slice(0, 64), 1))
                                if b1:
                                    s, f = row_taps(RHs + j1, 0, P1, PF,
                                                    add_c=not codd)
                                    seqs.append(s)
                                    if f:
                                        frows.append((j1, slice(64, 128), 0))
                                return dict(j0=j0, j1=j1, a0=a0, b0=b0,
                                            a1=a1, b1=b1, P0=P0, P1=P1,
                                            PF=PF, seqs=seqs, frows=frows,
                                            codd=codd)

                            def evac_pair(pr):
                                # x' = psum + C (alpha*x folded into tap 9):
                                # j0 + j1-A fused stt on DVE (psum + ch);
                                # j1-B Scalar copy then gpsimd SBUF add.
                                j0, j1 = pr["j0"], pr["j1"]
                                P0, P1, PF = pr["P0"], pr["P1"], pr["PF"]
                                if pr["a0"] and pr["b0"]:
                                    fstt(j0, slice(0, 128), P0)
                                elif pr["a0"]:
                                    fstt(j0, slice(0, 64), P0[0:64, :])
                                elif pr["b0"]:
                                    fstt(j0, slice(64, 128), P0[64:128, :])
                                if pr["codd"]:
                                    # odd groups: C tap rides a1 (quadrant
                                    # (0,64)) so the 11-tap load alternates
                                    # quadrants; evacs swap engines to match.
                                    if pr["a1"]:
                                        nc.vector.tensor_copy(
                                            out=xs[0:64, j1, 1:513],
                                            in_=P1[64:128, :])
                                    if pr["b1"]:
                                        fstt(j1, slice(64, 128), P1[0:64, :])
                                else:
                                    if pr["a1"]:
                                        fstt(j1, slice(0, 64), P1[64:128, :])
                                    if pr["b1"]:
                                        nc.scalar.activation(
                                            out=xs[64:128, j1, 1:513],
                                            in_=P1[0:64, :],
                                            func=AF.Copy, scale=1.0)
                                for (pj, xsl, phh) in pr["frows"]:
                                    fadd(pj, xsl,
                                         PF[phh * 64:phh * 64 + 64, :])

                            # round-robin taps across the 4 streams of each
                            # row-pair -> 4 concurrent PE quadrant streams.
                            for j0 in range(0, RHs, 2):
                                pr = build_pair(j0)
                                nmax = max((len(s) for s in pr["seqs"]),
                                           default=0)
                                for t in range(nmax):
                                    for s in pr["seqs"]:
                                        if t < len(s):
                                            nc.tensor.matmul(
                                                s[t]["out"], s[t]["lhsT"],
                                                s[t]["rhs"],
                                                start=s[t]["start"],
                                                stop=s[t]["stop"],
                                                skip_group_check=True,
                                                tile_position=s[t][
                                                    "tile_position"])
                                evac_pair(pr)
                                if k < T:
                                    while (len(thp_next) < NT
                                           and 4 * len(thp_next) + 3
                                           <= j0 + 1):
                                        thp_next.append(
                                            emit_tanh_tile(len(thp_next)))
                                elif (j0 % 4 == 2
                                      and len(nxt_thp) < nxt_NT):
                                    nxt_thp.append(
                                        nxt_emit(len(nxt_thp)))
                            if k < T:
                                while len(thp_next) < NT:
                                    thp_next.append(
                                        emit_tanh_tile(len(thp_next)))
                            else:
                                while len(nxt_thp) < nxt_NT:
                                    nxt_thp.append(nxt_emit(len(nxt_thp)))
                        # store valid rows (tile rows [T+1, Rs-T-1))
                        st_lo, st_hi = T + 1, Rs - (T + 1)
                        for blk in range(2):
                            lo, hi = blk * RHs, (blk + 1) * RHs
                            pr = slice(blk * 64, blk * 64 + 64)
                            s_lo, s_hi = max(lo, st_lo), min(hi, st_hi)
                            if s_lo >= s_hi:
                                continue
                            dst = x_out if last else dst_d
                            nc.sync.dma_start(
                                out=dst[:, base + s_lo:base + s_hi, :],
                                in_=xs[pr, s_lo - lo:s_hi - lo, :])
                    return nxt_thp

                # software pipeline: strip i+1's loads are emitted (and hence
                # enqueued on the sync/DMA engines) before strip i's compute,
                # so DMA overlaps compute via the bufs=2 tile rings.
                items = [(p, o0, hs) for p in range(1, NPASS + 1)
                         for (o0, hs) in g["STRIPS"]]
                def do_load(it):
                    r = strip_load(*it)
                    if r[2] is not None:
                        r[2]()
                    return r
                cur = do_load(items[0])
                thp1 = None
                for i, it in enumerate(items):
                    if i + 1 < len(items):
                        nxt_item = items[i + 1]
                        nxt_tiles = do_load(nxt_item)
                        nxt_RHs = (nxt_item[2] + 2 * T + 2) // 2
                        nxt = (nxt_tiles[0], nxt_RHs)
                    else:
                        nxt_tiles = nxt = None
                    thp1 = strip_compute(*it, cur[0], cur[1], thp1, nxt)
                    cur = nxt_tiles

    nc.compile()
    return nc


def host_prep(u, W_B, W_A, bias, alpha_logit, cfg):
    """Build per-core input maps. Only valid for the full-size problem."""
    g = _derive(cfg)
    SLAB, UROWS = g["SLAB"], g["UROWS"]
    B = u.shape[0]
    H = u.shape[2]
    Wc = 512

    alpha = np.float32(1.0 / (1.0 + np.exp(-np.float64(alpha_logit))))
    beta = np.float32(1.0) - alpha

    WAe = np.array(W_A, dtype=np.float32).copy()
    idx = np.arange(64)
    WAe[idx, idx, 1, 1] = np.maximum(WAe[idx, idx, 1, 1], np.float32(1.0))

    wa_taps = np.zeros((64, 11, 64), dtype=np.float32)
    for t9 in range(9):
        kh, kw = divmod(t9, 3)
        wa_taps[:, t9, :] = (beta * WAe[:, :, kh, kw]).T   # [cin, cout]
    wa_taps[:, 9, :] = alpha * np.eye(64, dtype=np.float32)
    wa_taps[:, 10, :] = np.eye(64, dtype=np.float32)
    wa_taps = wa_taps.astype(np.float16)

    bias_vec = np.array(bias, dtype=np.float32).reshape(64)
    wb9 = np.zeros((9, 64), dtype=np.float32)
    for kw in range(3):
        for kh in range(3):
            wb9[kw * 3 + kh, :] = W_B[:, 0, kh, kw]
    wb_hi = wb9.astype(ml_dtypes.bfloat16)
    wb_lo = (wb9 - wb_hi.astype(np.float32)).astype(ml_dtypes.bfloat16)
    wb27 = np.concatenate([wb_hi, wb_hi, wb_lo], axis=0)
    nbias = (beta * bias_vec).reshape(64, 1).astype(np.float32)
    mb = (-bias_vec).reshape(64, 1).astype(np.float32)
    alpha_arr = np.full((1, 1), alpha, dtype=np.float32)

    in_maps = []
    for core in range(8):
        b, h = divmod(core, 2)
        img = np.asarray(u[b, 0], dtype=np.float32)        # [H, 512]
        u_slab = np.zeros((UROWS, Wc), dtype=np.float32)
        if h == 0:
            # slab rows [-1, SLAB+1) = image rows [-1, SLAB+1)
            u_slab[1:UROWS] = img[0:SLAB + 1]
        else:
            off = H - SLAB                                  # 240
            # slab row s = image row s + off; u_in[j] = image j-1+off
            u_slab[0:UROWS - 1] = img[off - 1:H]
        u_hi = u_slab.astype(ml_dtypes.bfloat16)
        u_lo = (u_slab - u_hi.astype(np.float32)).astype(ml_dtypes.bfloat16)
        # im2col: u27[third*9 + kw*3 + kh, j, c] = src[j + kh, c + kw - 1]
        # (src rows are already the +-1 padded slab; cols zero-padded)
        u27 = np.zeros((27, SLAB, Wc), dtype=ml_dtypes.bfloat16)
        for third, src in enumerate((u_hi, u_lo, u_hi)):
            for kw in range(3):
                c_lo = max(0, 1 - kw)
                c_hi = min(Wc, Wc + 1 - kw)
                for kh in range(3):
                    t27 = third * 9 + kw * 3 + kh
                    u27[t27, :, c_lo:c_hi] = src[kh:kh + SLAB,
                                                 c_lo + kw - 1:c_hi + kw - 1]
        in_maps.append({
            "u27_in": u27,
            "wa_in": wa_taps,
            "wb_in": wb27,
            "nbias_in": nbias,
            "mb_in": mb,
            "alpha_in": alpha_arr,
        })
    return in_maps


_NC_CACHE = {}


def _get_nc(cfg_key=None):
    if "nc" not in _NC_CACHE:
        _NC_CACHE["nc"] = build(FULL_CFG)
    return _NC_CACHE["nc"]


def kernel(u, W_B, W_A, bias, alpha_logit, _trace=False):
    u = np.asarray(u, dtype=np.float32)
    B, _, H, Wc = u.shape
    nc = _get_nc()
    in_maps = host_prep(u, W_B, W_A, bias, alpha_logit, FULL_CFG)
    res = run_bass_kernel_spmd(nc, in_maps, core_ids=list(range(8)),
                               trace=_trace)
    SLAB = FULL_CFG["SLAB"]
    VALID = H // 2                                          # 256
    out = np.zeros((B, 64, H, Wc), dtype=np.float32)
    for core in range(8):
        b, h = divmod(core, 2)
        xo = res.results[core]["x_out"]                     # [64, SLAB, 514] f16
        xo = np.asarray(xo[:, :, 1:513], dtype=np.float32)
        if h == 0:
            out[b, :, 0:VALID, :] = xo[:, 0:VALID, :]
        else:
            out[b, :, VALID:H, :] = xo[:, SLAB - VALID:SLAB, :]
    kernel._last_results = res
    return out
